# revision 21
# baseline (speedup 1.0000x reference)
"""Trainium2 Bass kernel for nn_CenterModel — Chebyshev-basis formulation.

Computes -sum_w max_o ( C[w]*cos(o) - S[w]*sin(o) ) where
  C[w] = mean_n cos(2*pi*dist(n)/lambda[w]) * tid[n, w]
  S[w] = mean_n sin(2*pi*dist(n)/lambda[w]) * tid[n, w]

Key identity: cos(kappa_w * d) is analytic in d on [0, dmax], so expand in
Chebyshev polynomials of x = 2*d/dm - 1 (Jacobi-Anger, coefficients
~ J_m(z_w) with z_w = kappa_w*dm/2 <= ~44.5, decay super-exponentially
past m > z -> M = 48 basis functions measure rel_err ~ 6.5e-4):
  C[w] = (1/N) sum_m ac[m, w] * B[w, m],
  B[w, m] = sum_n tid[n, w] * T_m(x_n)    <- a true matmul over points.

Device schedule per core (62500 points, padded to 62592 = 489*128):
  - prologue: d = ||xy - center|| via Rsqrt + Newton (fp32)
  - Chebyshev recurrence T_m = E*T_{m-1} - T_{m-2} in fp16, split into 4
    j-chunks with separate SBUF tiles; chunk pairs interleaved to hide the
    serial-dependence stalls.  MMs for a j-chunk unlock as soon as that
    chunk's recurrence is done, overlapping the tid DMA stream.
  - tid loaded fp32->fp16 via SWDGE (cast inside the DMA engines)
  - per 128-point tile j: matmul(B_psum[128w, M], lhsT=tid16[:, j, :],
    rhs=T_chunk[:, :, j_local]) accumulating over all 489 tiles
  - ship B [128, M] fp32; host: all-reduce 8 cores, DCT coefficients,
    combine, tiny [W, 50] max.
Bottleneck: the 32 MB/core tid DMA (~95-105 us at ~358 GB/s/core).
"""

import os
import math
from contextlib import ExitStack

import numpy as np

import concourse.bacc as bacc
import concourse.bass as bass
import concourse.tile as tile
from concourse import mybir
from concourse.bass_utils import run_bass_kernel_spmd

F32 = mybir.dt.float32
F16 = mybir.dt.float16
AF = mybir.ActivationFunctionType
OP = mybir.AluOpType

N_POINTS = 500000
W = 128
N_OFFSETS = 50
N_CORES = 8
PER_CORE = N_POINTS // N_CORES  # 62500
NPP = 489                       # point-tiles per core (= points per partition)
N_PAD = NPP * 128               # 62592 padded rows per core
FD = 492                        # NPP padded: chunk widths even, offsets 4B-aligned
NT = int(os.environ.get("KERNEL_NT", "16"))  # point-tiles per DMA super-tile
TWO_PI = 2.0 * math.pi

# j-chunks of the recurrence: [start, width]; pairs (0,1) and (2,3) are
# emission-interleaved so the two serial chains hide each other's stalls.
CHUNKS = [(0, 120), (120, 120), (240, 126), (366, 126)]

M_DEFAULT = 48

_TID_LOAD = os.environ.get("KERNEL_TID_LOAD", "hwdge_scalar")
_TID_BUFS = int(os.environ.get("KERNEL_TID_BUFS", "20"))
_TID32_BUFS = int(os.environ.get("KERNEL_TID32_BUFS", "6"))
_NR_ITERS = int(os.environ.get("KERNEL_NR", "1"))

_cached = {}


def _build_program(m_basis):
    nc = bacc.Bacc(
        "TRN2",
        debug=False,
        enable_asserts=False,
        target_bir_lowering=False,
        num_devices=N_CORES,
    )
    xy_d = nc.dram_tensor("xy", [N_PAD, 2], F32, kind="ExternalInput")
    tid_d = nc.dram_tensor("tid", [N_PAD, W], F32, kind="ExternalInput")
    # host-replicated per-partition constants: [cx, cy, 4/dm, 0]
    cst_d = nc.dram_tensor("cst", [128, 4], F32, kind="ExternalInput")
    out_d = nc.dram_tensor("out", [128, m_basis], F32, kind="ExternalOutput")

    with tile.TileContext(nc) as tc, ExitStack() as ctx:
        consts = ctx.enter_context(tc.tile_pool(name="consts", bufs=1))
        tidp = ctx.enter_context(tc.tile_pool(name="tidp", bufs=_TID_BUFS))
        tid32p = ctx.enter_context(tc.tile_pool(name="tid32p", bufs=_TID32_BUFS))
        tmpp = ctx.enter_context(tc.tile_pool(name="tmpp", bufs=8))
        psump = ctx.enter_context(tc.tile_pool(name="psump", bufs=1, space="PSUM"))

        # ---------------- constants (host-replicated, plain DMA) ----------------
        cst = consts.tile([128, 4], F32)
        nc.sync.dma_start(out=cst, in_=cst_d[:, :])
        cen = cst
        scl = cst[:, 2:3]

        # ---------------- distances (fp32) ----------------
        xyf = consts.tile([128, NPP, 2], F32)
        nc.sync.dma_start(out=xyf, in_=xy_d[:, :].rearrange("(p j) c -> p j c", p=128))

        dx = consts.tile([128, NPP], F32)
        dy = consts.tile([128, NPP], F32)
        nc.vector.tensor_scalar(dx, xyf[:, :, 0], cen[:, 0:1], None, OP.subtract)
        nc.vector.tensor_scalar(dy, xyf[:, :, 1], cen[:, 1:2], None, OP.subtract)
        d2 = consts.tile([128, NPP], F32)
        dy2 = consts.tile([128, NPP], F32)
        nc.vector.tensor_tensor(d2, dx, dx, OP.mult)
        nc.vector.tensor_tensor(dy2, dy, dy, OP.mult)
        nc.vector.tensor_tensor(d2, d2, dy2, OP.add)
        nc.vector.tensor_scalar(d2, d2, 1e-12, None, OP.max)

        # d = sqrt(d2), Heron-refined with a fast approximate reciprocal:
        # s <- 0.5*(s + d2/s); error ~ eps_sqrt^2/2 + eps_recip/2 ~ 1e-5
        dall = consts.tile([128, NPP], F32)
        nc.scalar.activation(out=dall, in_=d2, func=AF.Sqrt)
        rcp = consts.tile([128, NPP], F32)
        t1 = consts.tile([128, NPP], F32)
        for _ in range(_NR_ITERS):
            nc.vector.reciprocal_approx_fast(out=rcp, in_=dall)
            nc.vector.tensor_tensor(t1, d2, rcp, OP.mult)
            nc.vector.tensor_tensor(t1, t1, dall, OP.add)
            nc.vector.tensor_scalar(dall, t1, 0.5, None, OP.mult)

        # ---------------- Chebyshev recurrence (fp16, 4 j-chunks) ----------------
        # E = 2x = d * (4/dm) - 2; tail cols (489..491) zeroed so chunk-D
        # T values stay in {0, +-1} there (no fp16 overflow; MMs never read them).
        E = consts.tile([128, FD], F16)
        nc.vector.memset(E[:, NPP:FD], 0.0)
        nc.vector.tensor_scalar(
            E[:, :NPP], dall, scl, 2.0, OP.mult, OP.subtract
        )

        # per-chunk T tiles, flattened 2D: T_ch[:, m*wd : (m+1)*wd]
        t_ch = [
            consts.tile([128, m_basis * wd], F16, name=f"t_ch{i}")
            for i, (_, wd) in enumerate(CHUNKS)
        ]

        def emit_pair(ia, ib):
            for i in (ia, ib):
                j0, wd = CHUNKS[i]
                nc.vector.memset(t_ch[i][:, 0:wd], 1.0)
            for i in (ia, ib):
                j0, wd = CHUNKS[i]
                nc.vector.tensor_scalar(
                    t_ch[i][:, wd : 2 * wd], E[:, j0 : j0 + wd], 0.5, None, OP.mult
                )
            for m in range(2, m_basis):
                us = {}
                for i in (ia, ib):
                    j0, wd = CHUNKS[i]
                    u = tmpp.tile([128, wd], F16, tag=f"u{i}")
                    nc.vector.tensor_tensor(
                        u, E[:, j0 : j0 + wd],
                        t_ch[i][:, (m - 1) * wd : m * wd], OP.mult,
                    )
                    us[i] = u
                for i in (ia, ib):
                    j0, wd = CHUNKS[i]
                    nc.vector.tensor_tensor(
                        t_ch[i][:, m * wd : (m + 1) * wd],
                        us[i], t_ch[i][:, (m - 2) * wd : (m - 1) * wd], OP.subtract,
                    )

        emit_pair(0, 1)
        emit_pair(2, 3)

        def rhs_for(j):
            for i, (j0, wd) in enumerate(CHUNKS):
                if j < j0 + wd:
                    jl = j - j0
                    sl = t_ch[i][:, jl : jl + 1]
                    # [128, m_basis] with free stride wd elements
                    return bass.AP(
                        tensor=sl.tensor,
                        offset=sl.offset,
                        ap=[list(sl.ap[0]), [wd, m_basis]],
                    )
            raise AssertionError(j)

        # ---------------- main loop: B[w, m] += tid_j^T @ T_j ----------------
        ps = psump.tile([128, m_basis], F32)
        tid_r = tid_d[:, :].rearrange("(p j) w -> p j w", p=128)
        n_super = (NPP + NT - 1) // NT
        for si in range(n_super):
            j0 = si * NT
            nt = min(NT, NPP - j0)
            tid_t = tidp.tile([128, NT, W], F16, tag="tid")
            if _TID_LOAD == "swdge" or (_TID_LOAD == "dual" and si % 2 == 0):
                nc.gpsimd.dma_start(
                    out=tid_t[:, :nt, :], in_=tid_r[:, j0 : j0 + nt, :]
                )
            else:
                # HWDGE fp32 load on the in-order sync queue + cast on the
                # otherwise-idle ScalarE (DVE stays free for the recurrence)
                tid32_t = tid32p.tile([128, NT, W], F32, tag="tid32")
                nc.sync.dma_start(
                    out=tid32_t[:, :nt, :], in_=tid_r[:, j0 : j0 + nt, :]
                )
                # ScalarE (otherwise idle) does every cast: it tracks the DMA
                # cadence (2.0us cast vs ~2.6us DMA) independent of when the
                # DVE recurrence finishes, keeping the tail robust
                if _TID_LOAD in ("hwdge_scalar", "dual"):
                    nc.scalar.copy(tid_t[:, :nt, :], tid32_t[:, :nt, :])
                else:
                    nc.vector.tensor_copy(tid_t[:, :nt, :], tid32_t[:, :nt, :])

            for t in range(nt):
                j = j0 + t
                nc.tensor.matmul(
                    ps,
                    lhsT=tid_t[:, t, :],
                    rhs=rhs_for(j),
                    start=(j == 0),
                    stop=(j == NPP - 1),
                )

        # ---------------- epilogue ----------------
        csum = consts.tile([128, m_basis], F32)
        nc.vector.tensor_copy(csum, ps)
        nc.sync.dma_start(out=out_d[:, :], in_=csum)

    nc.compile()
    return nc


def _get_program(m_basis):
    if m_basis not in _cached:
        _cached[m_basis] = _build_program(m_basis)
    return _cached[m_basis]


def _cheb_coeffs(kappa, dm, m_basis, n_nodes=2048):
    """Chebyshev coefficients of cos/sin(kappa_w * dm*(x+1)/2) on x in [-1,1]."""
    k = np.arange(n_nodes)
    x = np.cos(np.pi * (k + 0.5) / n_nodes)
    dd = dm * (x + 1.0) / 2.0
    ph = kappa[None, :] * dd[:, None]  # [K, W]
    cmat = np.cos(
        np.pi * np.arange(m_basis)[None, :] * (k[:, None] + 0.5) / n_nodes
    ) * (2.0 / n_nodes)  # [K, M]
    ac = cmat.T @ np.cos(ph)  # [M, W]
    as_ = cmat.T @ np.sin(ph)
    ac[0] *= 0.5
    as_[0] *= 0.5
    return ac, as_


# results of the last device run (for test harnesses to inspect timing)
last_run_results = None


def kernel(xy, tid, center, wavelength):
    global last_run_results

    xy = np.ascontiguousarray(np.asarray(xy), dtype=np.float32)
    tid = np.ascontiguousarray(np.asarray(tid), dtype=np.float32)
    center = np.ascontiguousarray(np.asarray(center), dtype=np.float32)
    wavelength = np.ascontiguousarray(np.asarray(wavelength), dtype=np.float32)

    d_host = np.sqrt(((xy.astype(np.float64) - center.astype(np.float64)) ** 2).sum(1))
    dmax = float(d_host.max())
    dm = dmax * 1.0005 + 1e-12
    kappa = TWO_PI / wavelength.astype(np.float64)
    zmax = kappa.max() * dm / 2.0
    m_basis = M_DEFAULT
    if zmax > 45.0:
        m_basis = int(-(-(zmax + 10.0 * (zmax / 2.0) ** (1.0 / 3.0) + 12.0) // 8) * 8)
    if "KERNEL_M" in os.environ:
        m_basis = int(os.environ["KERNEL_M"])

    nc = _get_program(m_basis)

    scale = np.array([4.0 / dm], dtype=np.float32)
    in_maps = []
    for c in range(N_CORES):
        lo = c * PER_CORE
        hi = lo + PER_CORE
        # pad rows get xy=center (x=-1, T bounded) and tid=0 (no contribution)
        xp = np.broadcast_to(center, (N_PAD, 2)).copy()
        xp[:PER_CORE] = xy[lo:hi]
        tp = np.zeros((N_PAD, W), dtype=np.float32)
        tp[:PER_CORE] = tid[lo:hi]
        cst = np.zeros((128, 4), dtype=np.float32)
        cst[:, 0] = center[0]
        cst[:, 1] = center[1]
        cst[:, 2] = scale[0]
        in_maps.append({"xy": xp, "tid": tp, "cst": cst})

    res = run_bass_kernel_spmd(
        nc,
        in_maps,
        list(range(N_CORES)),
        trace=bool(int(os.environ.get("KERNEL_TRACE", "0"))),
    )
    last_run_results = res

    B = np.zeros((W, m_basis), dtype=np.float64)
    for r in res.results:
        B += r["out"].astype(np.float64)

    ac, as_ = _cheb_coeffs(kappa, dm, m_basis)
    C = (B * ac.T).sum(axis=1) / N_POINTS  # [W]
    S = (B * as_.T).sum(axis=1) / N_POINTS

    offsets = np.linspace(0.0, TWO_PI, N_OFFSETS)
    vals = C[:, None] * np.cos(offsets)[None, :] - S[:, None] * np.sin(offsets)[None, :]
    metric = vals.max(axis=1).sum()
    return np.float32(-metric)


# revision 22
# speedup vs baseline: 1.1314x; 1.1314x over previous
"""Trainium2 Bass kernel for nn_CenterModel — Chebyshev-basis formulation.

Computes -sum_w max_o ( C[w]*cos(o) - S[w]*sin(o) ) where
  C[w] = mean_n cos(2*pi*dist(n)/lambda[w]) * tid[n, w]
  S[w] = mean_n sin(2*pi*dist(n)/lambda[w]) * tid[n, w]

Key identity: cos(kappa_w * d) is analytic in d on [0, dmax], so expand in
Chebyshev polynomials of x = 2*d/dm - 1 (Jacobi-Anger, coefficients
~ J_m(z_w) with z_w = kappa_w*dm/2 <= ~44.5, decay super-exponentially
past m > z -> M = 48 basis functions measure rel_err ~ 6.5e-4):
  C[w] = (1/N) sum_m ac[m, w] * B[w, m],
  B[w, m] = sum_n tid[n, w] * T_m(x_n)    <- a true matmul over points.

Device schedule per core (62500 points, padded to 62592 = 489*128):
  - prologue: d = ||xy - center|| via Rsqrt + Newton (fp32)
  - Chebyshev recurrence T_m = E*T_{m-1} - T_{m-2} in fp16, split into 4
    j-chunks with separate SBUF tiles; chunk pairs interleaved to hide the
    serial-dependence stalls.  MMs for a j-chunk unlock as soon as that
    chunk's recurrence is done, overlapping the tid DMA stream.
  - tid loaded fp32->fp16 via SWDGE (cast inside the DMA engines)
  - per 128-point tile j: matmul(B_psum[128w, M], lhsT=tid16[:, j, :],
    rhs=T_chunk[:, :, j_local]) accumulating over all 489 tiles
  - ship B [128, M] fp32; host: all-reduce 8 cores, DCT coefficients,
    combine, tiny [W, 50] max.
Bottleneck: the 32 MB/core tid DMA (~95-105 us at ~358 GB/s/core).
"""

import os
import math
from contextlib import ExitStack

import numpy as np

import concourse.bacc as bacc
import concourse.bass as bass
import concourse.tile as tile
from concourse import mybir
from concourse.bass_utils import run_bass_kernel_spmd

F32 = mybir.dt.float32
F16 = mybir.dt.float16
AF = mybir.ActivationFunctionType
OP = mybir.AluOpType

N_POINTS = 500000
W = 128
N_OFFSETS = 50
N_CORES = 8
PER_CORE = N_POINTS // N_CORES  # 62500
NPP = 489                       # point-tiles per core (= points per partition)
N_PAD = NPP * 128               # 62592 padded rows per core
FD = 492                        # NPP padded: chunk widths even, offsets 4B-aligned
NT = int(os.environ.get("KERNEL_NT", "16"))  # point-tiles per DMA super-tile
TWO_PI = 2.0 * math.pi

# j-chunks of the recurrence: [start, width]; pairs (0,1) and (2,3) are
# emission-interleaved so the two serial chains hide each other's stalls.
CHUNKS = [(0, 120), (120, 120), (240, 126), (366, 126)]

M_DEFAULT = 48

_TID_LOAD = os.environ.get("KERNEL_TID_LOAD", "hwdge_scalar")
_TID_BUFS = int(os.environ.get("KERNEL_TID_BUFS", "20"))
_TID32_BUFS = int(os.environ.get("KERNEL_TID32_BUFS", "6"))
_NR_ITERS = int(os.environ.get("KERNEL_NR", "1"))

_cached = {}


def _build_program(m_basis):
    nc = bacc.Bacc(
        "TRN2",
        debug=False,
        enable_asserts=False,
        target_bir_lowering=False,
        num_devices=N_CORES,
    )
    xy_d = nc.dram_tensor("xy", [N_PAD, 2], F32, kind="ExternalInput")
    tid_d = nc.dram_tensor("tid", [N_PAD, W], F32, kind="ExternalInput")
    # host-replicated per-partition constants: [cx, cy, 4/dm, 0]
    cst_d = nc.dram_tensor("cst", [128, 4], F32, kind="ExternalInput")
    out_d = nc.dram_tensor("out", [128, m_basis], F32, kind="ExternalOutput")

    with tile.TileContext(nc) as tc, ExitStack() as ctx:
        consts = ctx.enter_context(tc.tile_pool(name="consts", bufs=1))
        tidp = ctx.enter_context(tc.tile_pool(name="tidp", bufs=_TID_BUFS))
        tid32p = ctx.enter_context(tc.tile_pool(name="tid32p", bufs=_TID32_BUFS))
        tmpp = ctx.enter_context(tc.tile_pool(name="tmpp", bufs=8))
        psump = ctx.enter_context(tc.tile_pool(name="psump", bufs=1, space="PSUM"))

        # ---------------- constants (host-replicated, plain DMA) ----------------
        cst = consts.tile([128, 4], F32)
        nc.sync.dma_start(out=cst, in_=cst_d[:, :])
        cen = cst
        scl = cst[:, 2:3]

        # ---------------- distances (fp32) ----------------
        xyf = consts.tile([128, NPP, 2], F32)
        nc.sync.dma_start(out=xyf, in_=xy_d[:, :].rearrange("(p j) c -> p j c", p=128))

        dx = consts.tile([128, NPP], F32)
        dy = consts.tile([128, NPP], F32)
        nc.vector.tensor_scalar(dx, xyf[:, :, 0], cen[:, 0:1], None, OP.subtract)
        nc.vector.tensor_scalar(dy, xyf[:, :, 1], cen[:, 1:2], None, OP.subtract)
        d2 = consts.tile([128, NPP], F32)
        dy2 = consts.tile([128, NPP], F32)
        nc.vector.tensor_tensor(d2, dx, dx, OP.mult)
        nc.vector.tensor_tensor(dy2, dy, dy, OP.mult)
        nc.vector.tensor_tensor(d2, d2, dy2, OP.add)
        nc.vector.tensor_scalar(d2, d2, 1e-12, None, OP.max)

        # d = sqrt(d2), Heron-refined with a fast approximate reciprocal:
        # s <- 0.5*(s + d2/s); error ~ eps_sqrt^2/2 + eps_recip/2 ~ 1e-5
        dall = consts.tile([128, NPP], F32)
        nc.scalar.activation(out=dall, in_=d2, func=AF.Sqrt)
        rcp = consts.tile([128, NPP], F32)
        t1 = consts.tile([128, NPP], F32)
        for _ in range(_NR_ITERS):
            nc.vector.reciprocal_approx_fast(out=rcp, in_=dall)
            nc.vector.tensor_tensor(t1, d2, rcp, OP.mult)
            nc.vector.tensor_tensor(t1, t1, dall, OP.add)
            nc.vector.tensor_scalar(dall, t1, 0.5, None, OP.mult)

        # ---------------- Chebyshev recurrence (fp16, 4 j-chunks) ----------------
        # E = 2x = d * (4/dm) - 2; tail cols (489..491) zeroed so chunk-D
        # T values stay in {0, +-1} there (no fp16 overflow; MMs never read them).
        E = consts.tile([128, FD], F16)
        nc.vector.memset(E[:, NPP:FD], 0.0)
        nc.vector.tensor_scalar(
            E[:, :NPP], dall, scl, 2.0, OP.mult, OP.subtract
        )

        # per-chunk T tiles, flattened 2D: T_ch[:, m*wd : (m+1)*wd]
        t_ch = [
            consts.tile([128, m_basis * wd], F16, name=f"t_ch{i}")
            for i, (_, wd) in enumerate(CHUNKS)
        ]

        def emit_pair(ia, ib):
            for i in (ia, ib):
                j0, wd = CHUNKS[i]
                nc.vector.memset(t_ch[i][:, 0:wd], 1.0)
            for i in (ia, ib):
                j0, wd = CHUNKS[i]
                nc.vector.tensor_scalar(
                    t_ch[i][:, wd : 2 * wd], E[:, j0 : j0 + wd], 0.5, None, OP.mult
                )
            for m in range(2, m_basis):
                us = {}
                for i in (ia, ib):
                    j0, wd = CHUNKS[i]
                    u = tmpp.tile([128, wd], F16, tag=f"u{i}")
                    nc.vector.tensor_tensor(
                        u, E[:, j0 : j0 + wd],
                        t_ch[i][:, (m - 1) * wd : m * wd], OP.mult,
                    )
                    us[i] = u
                for i in (ia, ib):
                    j0, wd = CHUNKS[i]
                    nc.vector.tensor_tensor(
                        t_ch[i][:, m * wd : (m + 1) * wd],
                        us[i], t_ch[i][:, (m - 2) * wd : (m - 1) * wd], OP.subtract,
                    )

        emit_pair(0, 1)
        emit_pair(2, 3)

        def rhs_for(j):
            for i, (j0, wd) in enumerate(CHUNKS):
                if j < j0 + wd:
                    jl = j - j0
                    sl = t_ch[i][:, jl : jl + 1]
                    # [128, m_basis] with free stride wd elements
                    return bass.AP(
                        tensor=sl.tensor,
                        offset=sl.offset,
                        ap=[list(sl.ap[0]), [wd, m_basis]],
                    )
            raise AssertionError(j)

        # ---------------- main loop: B[w, m] += tid_j^T @ T_j ----------------
        ps = psump.tile([128, m_basis], F32)
        tid_r = tid_d[:, :].rearrange("(p j) w -> p j w", p=128)
        # uniform NT-tile chunks, then a shrinking tail (last chunks small) so
        # the post-DMA cast+MM tail on the critical path is minimal
        bounds = []
        j0 = 0
        while j0 < NPP:
            nt = min(NT, NPP - j0)
            if NPP - (j0 + nt) == 0 and nt > 4:
                bounds.append((j0, nt - 4))
                bounds.append((j0 + nt - 4, 3))
                bounds.append((j0 + nt - 1, 1))
                j0 += nt
            else:
                bounds.append((j0, nt))
                j0 += nt
        n_super = len(bounds)
        for si, (j0, nt) in enumerate(bounds):
            tid_t = tidp.tile([128, NT, W], F16, tag="tid")
            if _TID_LOAD == "swdge" or (_TID_LOAD == "dual" and si % 2 == 0):
                nc.gpsimd.dma_start(
                    out=tid_t[:, :nt, :], in_=tid_r[:, j0 : j0 + nt, :]
                )
            else:
                # HWDGE fp32 load on the in-order sync queue + cast on the
                # otherwise-idle ScalarE (DVE stays free for the recurrence)
                tid32_t = tid32p.tile([128, NT, W], F32, tag="tid32")
                nc.sync.dma_start(
                    out=tid32_t[:, :nt, :], in_=tid_r[:, j0 : j0 + nt, :]
                )
                # ScalarE (otherwise idle) does every cast: it tracks the DMA
                # cadence (2.0us cast vs ~2.6us DMA) independent of when the
                # DVE recurrence finishes, keeping the tail robust
                if _TID_LOAD in ("hwdge_scalar", "dual"):
                    nc.scalar.copy(tid_t[:, :nt, :], tid32_t[:, :nt, :])
                else:
                    nc.vector.tensor_copy(tid_t[:, :nt, :], tid32_t[:, :nt, :])

            for t in range(nt):
                j = j0 + t
                nc.tensor.matmul(
                    ps,
                    lhsT=tid_t[:, t, :],
                    rhs=rhs_for(j),
                    start=(j == 0),
                    stop=(j == NPP - 1),
                )

        # ---------------- epilogue ----------------
        csum = consts.tile([128, m_basis], F32)
        nc.vector.tensor_copy(csum, ps)
        nc.sync.dma_start(out=out_d[:, :], in_=csum)

    nc.compile()
    return nc


def _get_program(m_basis):
    if m_basis not in _cached:
        _cached[m_basis] = _build_program(m_basis)
    return _cached[m_basis]


def _cheb_coeffs(kappa, dm, m_basis, n_nodes=2048):
    """Chebyshev coefficients of cos/sin(kappa_w * dm*(x+1)/2) on x in [-1,1]."""
    k = np.arange(n_nodes)
    x = np.cos(np.pi * (k + 0.5) / n_nodes)
    dd = dm * (x + 1.0) / 2.0
    ph = kappa[None, :] * dd[:, None]  # [K, W]
    cmat = np.cos(
        np.pi * np.arange(m_basis)[None, :] * (k[:, None] + 0.5) / n_nodes
    ) * (2.0 / n_nodes)  # [K, M]
    ac = cmat.T @ np.cos(ph)  # [M, W]
    as_ = cmat.T @ np.sin(ph)
    ac[0] *= 0.5
    as_[0] *= 0.5
    return ac, as_


# results of the last device run (for test harnesses to inspect timing)
last_run_results = None


def kernel(xy, tid, center, wavelength):
    global last_run_results

    xy = np.ascontiguousarray(np.asarray(xy), dtype=np.float32)
    tid = np.ascontiguousarray(np.asarray(tid), dtype=np.float32)
    center = np.ascontiguousarray(np.asarray(center), dtype=np.float32)
    wavelength = np.ascontiguousarray(np.asarray(wavelength), dtype=np.float32)

    d_host = np.sqrt(((xy.astype(np.float64) - center.astype(np.float64)) ** 2).sum(1))
    dmax = float(d_host.max())
    dm = dmax * 1.0005 + 1e-12
    kappa = TWO_PI / wavelength.astype(np.float64)
    zmax = kappa.max() * dm / 2.0
    m_basis = M_DEFAULT
    if zmax > 45.0:
        m_basis = int(-(-(zmax + 10.0 * (zmax / 2.0) ** (1.0 / 3.0) + 12.0) // 8) * 8)
    if "KERNEL_M" in os.environ:
        m_basis = int(os.environ["KERNEL_M"])

    nc = _get_program(m_basis)

    scale = np.array([4.0 / dm], dtype=np.float32)
    in_maps = []
    for c in range(N_CORES):
        lo = c * PER_CORE
        hi = lo + PER_CORE
        # pad rows get xy=center (x=-1, T bounded) and tid=0 (no contribution)
        xp = np.broadcast_to(center, (N_PAD, 2)).copy()
        xp[:PER_CORE] = xy[lo:hi]
        tp = np.zeros((N_PAD, W), dtype=np.float32)
        tp[:PER_CORE] = tid[lo:hi]
        cst = np.zeros((128, 4), dtype=np.float32)
        cst[:, 0] = center[0]
        cst[:, 1] = center[1]
        cst[:, 2] = scale[0]
        in_maps.append({"xy": xp, "tid": tp, "cst": cst})

    res = run_bass_kernel_spmd(
        nc,
        in_maps,
        list(range(N_CORES)),
        trace=bool(int(os.environ.get("KERNEL_TRACE", "0"))),
    )
    last_run_results = res

    B = np.zeros((W, m_basis), dtype=np.float64)
    for r in res.results:
        B += r["out"].astype(np.float64)

    ac, as_ = _cheb_coeffs(kappa, dm, m_basis)
    C = (B * ac.T).sum(axis=1) / N_POINTS  # [W]
    S = (B * as_.T).sum(axis=1) / N_POINTS

    offsets = np.linspace(0.0, TWO_PI, N_OFFSETS)
    vals = C[:, None] * np.cos(offsets)[None, :] - S[:, None] * np.sin(offsets)[None, :]
    metric = vals.max(axis=1).sum()
    return np.float32(-metric)
